# revision 1
# baseline (speedup 1.0000x reference)
"""Trainium2 Bass kernel for DecomposingAttnProcessor (pad variant).

Math (pad branch contributes exactly zero since pad tokens are zeros
projected with no bias -> k_pad = v_pad = 0):
    q = hs @ Wq.T / (temp + eps)   (scale folded into Wq on host)
    k = ehs @ Wk.T ; v = ehs @ Wv.T
    scores[c,h,s,e] = q . k        (per head, dh=64)
    w = softmax over the 4 components c (dim 0)
    o = w @ v ; out = o @ Wo.T + bo + hs

Sharding: 8 cores, split S=4096 into 512-row blocks; all 4 components of
a block stay on one core (softmax couples them). K/V computed redundantly
per core (encoder seq is only 154).

Device layout is fully transposed (features on partitions): inputs are
host-transposed, output is produced transposed and host-untransposed.
"""

import numpy as np
import ml_dtypes

import concourse.bass as bass
import concourse.mybir as mybir
import concourse.tile as tile
from concourse import bacc
from concourse.bass_utils import run_bass_kernel_spmd

F32 = mybir.dt.float32
F32R = mybir.dt.float32r
BF16 = mybir.dt.bfloat16
AF = mybir.ActivationFunctionType
ALU = mybir.AluOpType

NCOMP = 4
HEADS = 24
DH = 64
D = 1536
S = 4096
E = 154
EPS = 1e-8
NCORES = 8
SL = S // NCORES          # 512 s-rows per core (per component)
SH = SL // 2              # 256: s-half tile width (N of most matmuls)
FT = D // 128             # 12 feature tiles of 128
HP = HEADS // 2           # 12 head-pairs (2 heads = 128 feature rows)
ECAT = NCOMP * E          # 616: components stacked along encoder axis
ETILES = ((0, 128), (128, E - 128))   # e split: 128 + 26


def _emit(tc):
    import os
    phases = os.environ.get("K_PHASES", "ABC")
    blevel = int(os.environ.get("K_BLEVEL", "4"))
    nc = tc.nc

    xT = nc.declare_dram_parameter("xT", [NCOMP, D, SL], F32, isOutput=False)
    xTb = nc.declare_dram_parameter("xTb", [NCOMP, D, SL], BF16, isOutput=False)
    eT = nc.declare_dram_parameter("eT", [D, ECAT], BF16, isOutput=False)
    wqT = nc.declare_dram_parameter("wqT", [D, D], BF16, isOutput=False)
    wkT = nc.declare_dram_parameter("wkT", [D, D], BF16, isOutput=False)
    wvT = nc.declare_dram_parameter("wvT", [D, D], BF16, isOutput=False)
    woT = nc.declare_dram_parameter("woT", [D, D], BF16, isOutput=False)
    bo = nc.declare_dram_parameter("bo", [128, FT], F32, isOutput=False)
    outT = nc.declare_dram_parameter("outT", [NCOMP, D, SL], F32, isOutput=True)

    # DRAM views with the 128-row tile index folded into the free dim, so a
    # whole [1536, n] panel loads as one DMA into a [128, FT*n] tile.
    xT_v = [xT[c].rearrange("(f p) s -> p f s", p=128) for c in range(NCOMP)]
    xTb_v = [xTb[c].rearrange("(f p) s -> p f s", p=128) for c in range(NCOMP)]
    eT_v = eT.rearrange("(f p) e -> p f e", p=128)
    wqT_v = wqT.rearrange("(f p) o -> p f o", p=128)
    wkT_v = wkT.rearrange("(f p) o -> p f o", p=128)
    wvT_v = wvT.rearrange("(f p) o -> p f o", p=128)
    woT_v = woT.rearrange("(f p) o -> p f o", p=128)
    outT_v = [outT[c].rearrange("(f p) s -> p f s", p=128) for c in range(NCOMP)]

    with tc.tile_pool(name="persist", bufs=1) as pp:
        # ---------------- persistent tiles ----------------
        kt_sb = [pp.tile([128, ECAT], BF16, tag="kT", bufs=FT, name=f"kt{t}")
                 for t in range(FT)]
        v_sb = [[pp.tile([esz, D], BF16, tag=f"v{ei}", bufs=NCOMP,
                         name=f"v{c}_{ei}")
                 for ei, (eo, esz) in enumerate(ETILES)] for c in range(NCOMP)]
        bo_sb = pp.tile([128, FT], F32, tag="bo", bufs=1, name="bo_sb")
        nc.sync.dma_start(out=bo_sb[:], in_=bo[:])

        def _phases():
            # ---------------- phase A: K^T and V ----------------
            if "A" in phases:
              with (
                tc.tile_pool(name="pha", bufs=1) as pa,
                tc.tile_pool(name="pha_psum", bufs=1, space="PSUM") as pap,
              ):
                et_b = pa.tile([128, FT * ECAT], BF16, tag="eT", bufs=1,
                               name="et_b")
                nc.sync.dma_start(
                    out=et_b.rearrange("p (f e) -> p f e", f=FT), in_=eT_v)
                et = [et_b[:, fi * ECAT:(fi + 1) * ECAT] for fi in range(FT)]

                # K^T[fo, c*E + e] over fi; N split 308+308 (>=256 keeps f32r
                # at full rate)
                for fot in range(FT):
                    wk_b = pa.tile([128, FT * 128], BF16, tag="wk", bufs=3,
                                   name=f"wk{fot}")
                    nc.sync.dma_start(
                        out=wk_b.rearrange("p (f o) -> p f o", f=FT),
                        in_=wkT_v[:, :, fot * 128:(fot + 1) * 128])
                    for nch in range(2):
                        n0 = nch * 308
                        pk = pap.tile([128, 308], F32, tag="pk", bufs=2,
                                      name=f"pk{fot}_{nch}")
                        for fi in range(FT):
                            nc.tensor.matmul(
                                pk[:], wk_b[:, fi * 128:(fi + 1) * 128],
                                et[fi][:, n0:n0 + 308],
                                start=(fi == 0), stop=(fi == FT - 1))
                        nc.vector.tensor_copy(
                            out=kt_sb[fot][:, n0:n0 + 308], in_=pk[:])

                # V[c][e, fv] (natural layout, bf16) over fi
                for fvc in range(3):
                    wv_b = pa.tile([128, FT * 512], BF16, tag="wv", bufs=2,
                                   name=f"wv{fvc}")
                    nc.sync.dma_start(
                        out=wv_b.rearrange("p (f o) -> p f o", f=FT),
                        in_=wvT_v[:, :, fvc * 512:(fvc + 1) * 512])
                    for c in range(NCOMP):
                        for ei, (eo, esz) in enumerate(ETILES):
                            pv = pap.tile([128, 512], F32, tag="pv", bufs=2,
                                          name=f"pv{fvc}_{c}_{ei}")
                            for fi in range(FT):
                                nc.tensor.matmul(
                                    pv[:esz, :],
                                    et[fi][:, c * E + eo:c * E + eo + esz],
                                    wv_b[:, fi * 512:(fi + 1) * 512],
                                    start=(fi == 0), stop=(fi == FT - 1))
                            nc.vector.tensor_copy(
                                out=v_sb[c][ei][:, fvc * 512:(fvc + 1) * 512],
                                in_=pv[:esz, :])

            # ---------------- phases B+C per s-half ----------------
            with (
                tc.tile_pool(name="bc", bufs=1) as bc,
                tc.tile_pool(name="bcp", bufs=1, space="PSUM") as bcp,
            ):
                for half in range(2):
                    s0 = half * SH
                    # bf16 x^T panels for the Q projection
                    xh = []
                    for c in range(NCOMP):
                        t = bc.tile([128, FT * SH], BF16, tag="xh", bufs=5,
                                    name=f"xh{half}_{c}")
                        nc.sync.dma_start(
                            out=t.rearrange("p (f s) -> p f s", f=FT),
                            in_=xTb_v[c][:, :, s0:s0 + SH])
                        xh.append(t)

                    # -------- phase B: Q, scores, softmax, o --------
                    ot_sb = {}
                    for hp in range(HP if "B" in phases else 0):
                        wq_b = bc.tile([128, FT * 128], BF16, tag="wq", bufs=2,
                                       name=f"wq{half}_{hp}")
                        nc.sync.dma_start(
                            out=wq_b.rearrange("p (f o) -> p f o", f=FT),
                            in_=wqT_v[:, :, hp * 128:(hp + 1) * 128])

                        # Q^T for the two heads of this pair, all 4 components
                        qt = []
                        for c in range(NCOMP):
                            pq = bcp.tile([128, SH], F32, tag="pq", bufs=2,
                                          name=f"pq{half}_{hp}_{c}")
                            for fi in range(FT):
                                nc.tensor.matmul(
                                    pq[:], wq_b[:, fi * 128:(fi + 1) * 128],
                                    xh[c][:, fi * SH:(fi + 1) * SH],
                                    start=(fi == 0), stop=(fi == FT - 1))
                            q = bc.tile([128, SH], BF16, tag="qT", bufs=6,
                                        name=f"qt{half}_{hp}_{c}")
                            nc.scalar.copy(q[:], pq[:])
                            qt.append(q)
                        if blevel < 2:
                            continue

                        # scores + exp, both heads packed along the free dim
                        exps = [[None, None] for _ in range(NCOMP)]
                        for ei, (eo, esz) in enumerate(ETILES):
                            for c in range(NCOMP):
                                # separate psum banks per head: matmul psum
                                # writes must start at a bank boundary
                                ex = bc.tile([esz, 2 * SH], BF16, tag=f"exp{ei}",
                                             bufs=6, name=f"ex{half}_{hp}_{ei}_{c}")
                                for hh in range(2):
                                    ps = bcp.tile([128, SH], F32, tag="ps",
                                                  bufs=3,
                                                  name=f"ps{half}_{hp}_{ei}_{c}_{hh}")
                                    nc.tensor.matmul(
                                        ps[:esz, :],
                                        kt_sb[hp][hh * 64:(hh + 1) * 64,
                                                  c * E + eo:c * E + eo + esz],
                                        qt[c][hh * 64:(hh + 1) * 64, :],
                                        start=True, stop=True)
                                    nc.scalar.activation(
                                        ex[:, hh * SH:(hh + 1) * SH],
                                        ps[:esz, :], AF.Exp)
                                exps[c][ei] = ex
                            ssum = bc.tile([esz, 2 * SH], BF16, tag=f"sum{ei}",
                                           bufs=4, name=f"sm{half}_{hp}_{ei}")
                            nc.vector.tensor_add(out=ssum[:], in0=exps[0][ei][:],
                                                 in1=exps[1][ei][:])
                            nc.vector.tensor_add(out=ssum[:], in0=ssum[:],
                                                 in1=exps[2][ei][:])
                            nc.vector.tensor_add(out=ssum[:], in0=ssum[:],
                                                 in1=exps[3][ei][:])
                            rinv = bc.tile([esz, 2 * SH], BF16, tag=f"sum{ei}",
                                           bufs=4, name=f"ri{half}_{hp}_{ei}")
                            with nc.allow_low_precision(
                                    reason="softmax weights are consumed in bf16"):
                                nc.vector.reciprocal(out=rinv[:], in_=ssum[:])
                            for c in range(NCOMP):
                                w = bc.tile([esz, 2 * SH], BF16, tag=f"w{ei}",
                                            bufs=6, name=f"w{half}_{hp}_{ei}_{c}")
                                nc.vector.tensor_mul(out=w[:], in0=exps[c][ei][:],
                                                     in1=rinv[:])
                                exps[c][ei] = w  # normalized weights

                        # o^T: V-slices @ w; head hh lands on psum partitions
                        # hh*64..hh*64+64 (own accumulation group per head, both
                        # column-aligned to the bank start)
                        for c in range(NCOMP if blevel >= 4 else 0):
                            po = bcp.tile([128, SH], F32, tag="po", bufs=2,
                                          name=f"po{half}_{hp}_{c}")
                            for hh in range(2):
                                h = hp * 2 + hh
                                for ei, (eo, esz) in enumerate(ETILES):
                                    nc.tensor.matmul(
                                        po[hh * 64:(hh + 1) * 64, :],
                                        v_sb[c][ei][:, h * 64:(h + 1) * 64],
                                        exps[c][ei][:, hh * SH:(hh + 1) * SH],
                                        start=(ei == 0), stop=(ei == 1),
                                        skip_group_check=True)
                            ot = bc.tile([128, SH], BF16, tag="oT", bufs=48,
                                         name=f"ot{half}_{hp}_{c}")
                            nc.vector.tensor_copy(out=ot[:], in_=po[:])
                            ot_sb[(c, hp)] = ot

                    # -------- phase C: out-proj + bias + residual --------
                    for fot in range(FT if "C" in phases else 0):
                        wo_b = bc.tile([128, FT * 128], BF16, tag="wo", bufs=3,
                                       name=f"wo{half}_{fot}")
                        nc.sync.dma_start(
                            out=wo_b.rearrange("p (f o) -> p f o", f=FT),
                            in_=woT_v[:, :, fot * 128:(fot + 1) * 128])
                        for c in range(NCOMP):
                            xr = bc.tile([128, SH], F32, tag="xr", bufs=4,
                                         name=f"xr{half}_{fot}_{c}")
                            nc.sync.dma_start(
                                out=xr[:],
                                in_=xT_v[c][:, fot, s0:s0 + SH])
                            po = bcp.tile([128, SH], F32, tag="pout", bufs=1,
                                          name=f"pc{half}_{fot}_{c}")
                            for fi in range(FT):
                                nc.tensor.matmul(
                                    po[:], wo_b[:, fi * 128:(fi + 1) * 128],
                                    ot_sb[(c, fi)][:],
                                    start=(fi == 0), stop=(fi == FT - 1))
                            ob = bc.tile([128, SH], F32, tag="outsb", bufs=4,
                                         name=f"ob{half}_{fot}_{c}")
                            nc.vector.scalar_tensor_tensor(
                                out=ob[:], in0=po[:],
                                scalar=bo_sb[:, fot:fot + 1],
                                in1=xr[:],
                                op0=ALU.add, op1=ALU.add)
                            nc.sync.dma_start(
                                out=outT_v[c][:, fot, s0:s0 + SH], in_=ob[:])


        repeat = int(os.environ.get("K_REPEAT", "1"))
        for _rep in range(repeat):
            _phases()


_NC_CACHE = {}


def _get_nc():
    if "nc" not in _NC_CACHE:
        nc = bacc.Bacc("TRN2", target_bir_lowering=False)
        with tile.TileContext(nc) as tc:
            _emit(tc)
        nc.compile()
        _NC_CACHE["nc"] = nc
    return _NC_CACHE["nc"]


def kernel(hidden_states, encoder_hidden_states, temperature, Wq, Wk, Wv, Wo,
           bo, pad_length):
    # pad branch contributes zero to the output (zeros projected with no
    # bias give k_pad = v_pad = 0), so pad_length is irrelevant.
    hs = np.ascontiguousarray(np.asarray(hidden_states, dtype=np.float32))
    ehs = np.ascontiguousarray(
        np.asarray(encoder_hidden_states, dtype=np.float32))
    temp = float(np.asarray(temperature).reshape(-1)[0])
    Wq = np.asarray(Wq, dtype=np.float32)
    Wk = np.asarray(Wk, dtype=np.float32)
    Wv = np.asarray(Wv, dtype=np.float32)
    Wo = np.asarray(Wo, dtype=np.float32)
    bo_v = np.asarray(bo, dtype=np.float32).reshape(-1)

    wqT = np.ascontiguousarray((Wq / (temp + EPS)).T).astype(ml_dtypes.bfloat16)
    wkT = np.ascontiguousarray(Wk.T).astype(ml_dtypes.bfloat16)
    wvT = np.ascontiguousarray(Wv.T).astype(ml_dtypes.bfloat16)
    woT = np.ascontiguousarray(Wo.T).astype(ml_dtypes.bfloat16)
    eT_all = np.ascontiguousarray(
        np.concatenate([ehs[c].T for c in range(NCOMP)],
                       axis=1)).astype(ml_dtypes.bfloat16)
    bo_t = np.ascontiguousarray(bo_v.reshape(FT, 128).T)

    nc = _get_nc()
    in_maps = []
    for i in range(NCORES):
        xT_i = np.ascontiguousarray(
            hs[:, i * SL:(i + 1) * SL, :].transpose(0, 2, 1))
        in_maps.append({
            "xT": xT_i, "xTb": xT_i.astype(ml_dtypes.bfloat16),
            "eT": eT_all, "wqT": wqT, "wkT": wkT,
            "wvT": wvT, "woT": woT, "bo": bo_t,
        })

    res = run_bass_kernel_spmd(nc, in_maps, core_ids=list(range(NCORES)))

    out = np.empty((NCOMP, S, D), dtype=np.float32)
    for i in range(NCORES):
        out[:, i * SL:(i + 1) * SL, :] = res.results[i]["outT"].transpose(
            0, 2, 1)
    return out



# revision 9
# speedup vs baseline: 1.2475x; 1.2475x over previous
"""Trainium2 Bass kernel for DecomposingAttnProcessor (pad variant), v2.

Math (pad branch contributes exactly zero since pad tokens are zeros
projected with no bias -> k_pad = v_pad = 0):
    q = hs @ Wq.T / (temp + eps)   (scale folded into Wq on host)
    k = ehs @ Wk.T ; v = ehs @ Wv.T
    scores[c,h,s,e] = q . k        (per head, dh=64)
    w = softmax over the 4 components c (dim 0)
    o = w @ v ; out = o @ Wo.T + bo + hs

Sharding: 8 cores, split S=4096 into 512-row blocks; all 4 components of
a block stay on one core (softmax couples them). K/V computed redundantly
per core (encoder seq is only 154).

v2 design vs baseline:
 - one full 512-column pass over the s-block (N=512 matmuls, Wq/Wo one load)
 - E padded to 160 per component host-side (zeros); e-tails of the 4
   components packed at 32-partition offsets of shared tiles so the tail
   score/V matmuls go to one PSUM bank (concurrent via tile_position) and
   tail softmax runs at 1/4 the instruction count
 - softmax: exp on ScalarE, sums on DVE (f32), reciprocal_approx_fast,
   weight muls on GpSimd (main) / DVE (tails)
 - o-matmuls of hp-1 emitted after scores of hp (SW pipeline) so the
   in-order tensor stream never stalls on the softmax chain
 - all DRAM tensors host-arranged so every DMA is dense per partition
"""

import numpy as np
import ml_dtypes

import concourse.bass as bass
import concourse.mybir as mybir
import concourse.tile as tile
from concourse import bacc
from concourse.bass_utils import run_bass_kernel_spmd

F32 = mybir.dt.float32
BF16 = mybir.dt.bfloat16
AF = mybir.ActivationFunctionType
ALU = mybir.AluOpType

NCOMP = 4
HEADS = 24
DH = 64
D = 1536
S = 4096
E = 154
EP = 160                  # padded per-component encoder length
ECP = NCOMP * EP          # 640
EPS = 1e-8
NCORES = 8
SL = S // NCORES          # 512 s-rows per core (per component)
FT = D // 128             # 12 feature tiles of 128
HP = HEADS // 2           # 12 head-pairs (2 heads = 128 feature rows)


def _emit(tc):
    nc = tc.nc

    x = nc.declare_dram_parameter("x", [NCOMP, 128, FT * SL], BF16,
                                  isOutput=False)
    eT = nc.declare_dram_parameter("eT", [128, FT * ECP], BF16, isOutput=False)
    wq = nc.declare_dram_parameter("wq", [FT, 128, FT * 128], BF16,
                                   isOutput=False)
    wk = nc.declare_dram_parameter("wk", [FT, 128, FT * 128], BF16,
                                   isOutput=False)
    wv = nc.declare_dram_parameter("wv", [3, 128, FT * 512], BF16,
                                   isOutput=False)
    wo = nc.declare_dram_parameter("wo", [FT, 128, FT * 128], BF16,
                                   isOutput=False)
    bo = nc.declare_dram_parameter("bo", [128, FT], F32, isOutput=False)
    outT = nc.declare_dram_parameter("outT", [NCOMP, 128, FT * SL], F32,
                                     isOutput=True)

    with tc.tile_pool(name="persist", bufs=1) as pp:
        # ---------------- persistent tiles ----------------
        kt_sb = [pp.tile([128, ECP], BF16, tag="kT", bufs=FT, name=f"kt{t}")
                 for t in range(FT)]
        v_sb = [pp.tile([128, D], BF16, tag="v", bufs=NCOMP, name=f"v{c}")
                for c in range(NCOMP)]
        vt_sb = [pp.tile([32, D], BF16, tag="vt", bufs=NCOMP, name=f"vt{c}")
                 for c in range(NCOMP)]
        x_sb = [pp.tile([128, FT * SL], BF16, tag="x", bufs=NCOMP,
                        name=f"x{c}")
                for c in range(NCOMP)]
        bo_sb = pp.tile([128, FT], F32, tag="bo", bufs=1, name="bo_sb")

        nc.sync.dma_start(out=bo_sb[:], in_=bo[:])
        for c in range(NCOMP):
            nc.sync.dma_start(out=x_sb[c][:], in_=x[c])

        # ---------------- phase A: K^T and V ----------------
        with (
            tc.tile_pool(name="pha", bufs=1) as pa,
            tc.tile_pool(name="pha_psum", bufs=1, space="PSUM") as pap,
        ):
            et_b = pa.tile([128, FT * ECP], BF16, tag="eT", bufs=1,
                           name="et_b")
            nc.sync.dma_start(out=et_b[:], in_=eT[:])

            # K^T[fo, c*EP + e] over fi; N split 320+320
            for fot in range(FT):
                wk_b = pa.tile([128, FT * 128], BF16, tag="wk", bufs=3,
                               name=f"wk{fot}")
                nc.sync.dma_start(out=wk_b[:], in_=wk[fot])
                for nch in range(2):
                    n0 = nch * 320
                    pk = pap.tile([128, 320], F32, tag="pk", bufs=2,
                                  name=f"pk{fot}_{nch}")
                    for fi in range(FT):
                        nc.tensor.matmul(
                            pk[:], wk_b[:, fi * 128:(fi + 1) * 128],
                            et_b[:, fi * ECP + n0:fi * ECP + n0 + 320],
                            start=(fi == 0), stop=(fi == FT - 1))
                    nc.vector.tensor_copy(
                        out=kt_sb[fot][:, n0:n0 + 320], in_=pk[:])

            # V[c][e, fv] (e on partitions); tails of the 4 comps packed at
            # partition offsets 32c of one shared tile (pad rows are zero
            # because the eT pad columns are zero)
            for fvc in range(3):
                wv_b = pa.tile([128, FT * 512], BF16, tag="wv", bufs=2,
                               name=f"wv{fvc}")
                nc.sync.dma_start(out=wv_b[:], in_=wv[fvc])
                for c in range(NCOMP):
                    pv = pap.tile([128, 512], F32, tag="pv", bufs=2,
                                  name=f"pv{fvc}_{c}")
                    for fi in range(FT):
                        nc.tensor.matmul(
                            pv[:],
                            et_b[:, fi * ECP + c * EP:fi * ECP + c * EP + 128],
                            wv_b[:, fi * 512:(fi + 1) * 512],
                            start=(fi == 0), stop=(fi == FT - 1))
                    nc.vector.tensor_copy(
                        out=v_sb[c][:, fvc * 512:(fvc + 1) * 512], in_=pv[:])
                for c in range(NCOMP):
                    pvt = pap.tile([32, 512], F32, tag="pv", bufs=2,
                                   name=f"pvt{fvc}_{c}")
                    for fi in range(FT):
                        nc.tensor.matmul(
                            pvt[:],
                            et_b[:, fi * ECP + c * EP + 128:
                                 fi * ECP + c * EP + 160],
                            wv_b[:, fi * 512:(fi + 1) * 512],
                            start=(fi == 0), stop=(fi == FT - 1))
                    nc.vector.tensor_copy(
                        out=vt_sb[c][:, fvc * 512:(fvc + 1) * 512], in_=pvt[:])

        # ---------------- phases B+C ----------------
        with (
            tc.tile_pool(name="bc", bufs=1) as bc,
            tc.tile_pool(name="bcp", bufs=1, space="PSUM") as bcp,
        ):
            ot_sb = {}

            def emit_o(hp, wex, wt):
                """o-matmuls + psum evacuation for head-pair hp."""
                for c in range(NCOMP):
                    po = bcp.tile([128, 512], F32, tag="po", bufs=2,
                                  name=f"po{hp}_{c}")
                    for hh in range(2):
                        h = hp * 2 + hh
                        nc.tensor.matmul(
                            po[hh * 64:(hh + 1) * 64, :],
                            v_sb[c][:, h * 64:(h + 1) * 64],
                            wex[c][:, hh * 512:(hh + 1) * 512],
                            start=True, stop=False, skip_group_check=True)
                        nc.tensor.matmul(
                            po[hh * 64:(hh + 1) * 64, :],
                            vt_sb[c][:, h * 64:(h + 1) * 64],
                            wt[c][:, hh * 512:(hh + 1) * 512],
                            start=False, stop=True, skip_group_check=True)
                    ot = bc.tile([128, 512], BF16, tag="ot", bufs=48,
                                 name=f"ot{hp}_{c}")
                    nc.vector.tensor_copy(out=ot[:], in_=po[:])
                    ot_sb[(c, hp)] = ot

            prev = None
            for hp in range(HP):
                wq_b = bc.tile([128, FT * 128], BF16, tag="wq", bufs=2,
                               name=f"wq{hp}")
                nc.sync.dma_start(out=wq_b[:], in_=wq[hp])

                # Q^T for the two heads of this pair, all 4 components
                qt = []
                for c in range(NCOMP):
                    pq = bcp.tile([128, 512], F32, tag="pq", bufs=2,
                                  name=f"pq{hp}_{c}")
                    for fi in range(FT):
                        nc.tensor.matmul(
                            pq[:], wq_b[:, fi * 128:(fi + 1) * 128],
                            x_sb[c][:, fi * 512:(fi + 1) * 512],
                            start=(fi == 0), stop=(fi == FT - 1))
                    q = bc.tile([128, 512], BF16, tag="qt", bufs=4,
                                name=f"qt{hp}_{c}")
                    nc.vector.tensor_copy(out=q[:], in_=pq[:])
                    qt.append(q)

                # scores + exp; the two heads pack the free dim of ex tiles
                ex = []
                for c in range(NCOMP):
                    e = bc.tile([128, 1024], BF16, tag="ex", bufs=5,
                                name=f"ex{hp}_{c}")
                    for hh in range(2):
                        ps = bcp.tile([128, 512], F32, tag="ps", bufs=4,
                                      name=f"ps{hp}_{c}_{hh}")
                        nc.tensor.matmul(
                            ps[:],
                            kt_sb[hp][hh * 64:(hh + 1) * 64,
                                      c * EP:c * EP + 128],
                            qt[c][hh * 64:(hh + 1) * 64, :],
                            start=True, stop=True)
                        nc.scalar.activation(
                            e[:, hh * 512:(hh + 1) * 512], ps[:], AF.Exp)
                    ex.append(e)
                # tails: per-comp [32, s] tiles (rows 26..32 are zero-pad)
                ext = []
                for c in range(NCOMP):
                    et_t = bc.tile([32, 1024], BF16, tag="ext", bufs=4,
                                   name=f"ext{hp}_{c}")
                    for hh in range(2):
                        pst = bcp.tile([32, 512], F32, tag="ps", bufs=4,
                                       name=f"pst{hp}_{c}_{hh}")
                        nc.tensor.matmul(
                            pst[:],
                            kt_sb[hp][hh * 64:(hh + 1) * 64,
                                      c * EP + 128:c * EP + 160],
                            qt[c][hh * 64:(hh + 1) * 64, :],
                            start=True, stop=True)
                        nc.scalar.activation(
                            et_t[:, hh * 512:(hh + 1) * 512], pst[:], AF.Exp)
                    ext.append(et_t)

                # softmax over the 4 components
                s01 = bc.tile([128, 1024], BF16, tag="ssum", bufs=3,
                              name=f"s01_{hp}")
                nc.vector.tensor_add(out=s01[:], in0=ex[0][:], in1=ex[1][:])
                s23 = bc.tile([128, 1024], BF16, tag="ssum", bufs=3,
                              name=f"s23_{hp}")
                nc.vector.tensor_add(out=s23[:], in0=ex[2][:], in1=ex[3][:])
                sf = bc.tile([128, 1024], F32, tag="sf", bufs=1,
                             name=f"sf{hp}")
                nc.vector.tensor_add(out=sf[:], in0=s01[:], in1=s23[:])
                rf = bc.tile([128, 1024], F32, tag="rf", bufs=1,
                             name=f"rf{hp}")
                nc.vector.reciprocal_approx_fast(out=rf[:], in_=sf[:])
                rb = bc.tile([128, 1024], BF16, tag="rb", bufs=2,
                             name=f"rb{hp}")
                nc.vector.tensor_copy(out=rb[:], in_=rf[:])

                st1 = bc.tile([32, 1024], BF16, tag="ssum", bufs=3,
                              name=f"st1_{hp}")
                nc.vector.tensor_add(out=st1[:], in0=ext[0][:], in1=ext[1][:])
                st2 = bc.tile([32, 1024], BF16, tag="ssum", bufs=3,
                              name=f"st2_{hp}")
                nc.vector.tensor_add(out=st2[:], in0=ext[2][:], in1=ext[3][:])
                stf = bc.tile([32, 1024], F32, tag="sf", bufs=1,
                              name=f"stf{hp}")
                nc.vector.tensor_add(out=stf[:], in0=st1[:], in1=st2[:])
                rtf = bc.tile([32, 1024], F32, tag="rf", bufs=1,
                              name=f"rtf{hp}")
                nc.vector.reciprocal_approx_fast(out=rtf[:], in_=stf[:])
                rtb = bc.tile([32, 1024], BF16, tag="rb", bufs=2,
                              name=f"rtb{hp}")
                nc.vector.tensor_copy(out=rtb[:], in_=rtf[:])

                # normalized weights: main on GpSimd, tails on DVE
                wex = []
                for c in range(NCOMP):
                    w = bc.tile([128, 1024], BF16, tag="wex", bufs=5,
                                name=f"wex{hp}_{c}")
                    nc.gpsimd.tensor_mul(out=w[:], in0=ex[c][:], in1=rb[:])
                    wex.append(w)
                wt = []
                for c in range(NCOMP):
                    w = bc.tile([32, 1024], BF16, tag="wt", bufs=4,
                                name=f"wt{hp}_{c}")
                    nc.gpsimd.tensor_mul(out=w[:], in0=ext[c][:], in1=rtb[:])
                    wt.append(w)

                # SW pipeline: previous hp's o-matmuls issue after this hp's
                # Q/scores so the tensor stream doesn't stall on softmax
                if prev is not None:
                    emit_o(*prev)
                prev = (hp, wex, wt)
            emit_o(*prev)

            # -------- phase C: out-proj + bias + residual --------
            for fot in range(FT):
                wo_b = bc.tile([128, FT * 128], BF16, tag="wq", bufs=2,
                               name=f"wo{fot}")
                nc.sync.dma_start(out=wo_b[:], in_=wo[fot])
                for c in range(NCOMP):
                    pco = bcp.tile([128, 512], F32, tag="po", bufs=2,
                                   name=f"pc{fot}_{c}")
                    for fi in range(FT):
                        nc.tensor.matmul(
                            pco[:], wo_b[:, fi * 128:(fi + 1) * 128],
                            ot_sb[(c, fi)][:],
                            start=(fi == 0), stop=(fi == FT - 1))
                    ob = bc.tile([128, 512], F32, tag="ob", bufs=4,
                                 name=f"ob{fot}_{c}")
                    nc.vector.scalar_tensor_tensor(
                        out=ob[:], in0=pco[:],
                        scalar=bo_sb[:, fot:fot + 1],
                        in1=x_sb[c][:, fot * 512:(fot + 1) * 512],
                        op0=ALU.add, op1=ALU.add)
                    nc.sync.dma_start(
                        out=outT[c][:, fot * 512:(fot + 1) * 512], in_=ob[:])


_NC_CACHE = {}


def _get_nc():
    if "nc" not in _NC_CACHE:
        nc = bacc.Bacc("TRN2", target_bir_lowering=False)
        with tile.TileContext(nc) as tc:
            _emit(tc)
        nc.compile()
        _NC_CACHE["nc"] = nc
    return _NC_CACHE["nc"]


def _panels_oxo(wT, nblk, blk):
    """[D, D] -> [nblk, 128, FT*blk]: panel[b][p][fi*blk+o] = wT[fi*128+p, b*blk+o]."""
    return np.ascontiguousarray(
        wT.reshape(FT, 128, nblk, blk).transpose(2, 1, 0, 3)
        .reshape(nblk, 128, FT * blk)).astype(ml_dtypes.bfloat16)


def kernel(hidden_states, encoder_hidden_states, temperature, Wq, Wk, Wv, Wo,
           bo, pad_length):
    # pad branch contributes zero to the output (zeros projected with no
    # bias give k_pad = v_pad = 0), so pad_length is irrelevant.
    hs = np.asarray(hidden_states, dtype=np.float32)
    ehs = np.asarray(encoder_hidden_states, dtype=np.float32)
    temp = float(np.asarray(temperature).reshape(-1)[0])
    Wq = np.asarray(Wq, dtype=np.float32)
    Wk = np.asarray(Wk, dtype=np.float32)
    Wv = np.asarray(Wv, dtype=np.float32)
    Wo = np.asarray(Wo, dtype=np.float32)
    bo_v = np.asarray(bo, dtype=np.float32).reshape(-1)

    wq_p = _panels_oxo((Wq / (temp + EPS)).T, FT, 128)
    wk_p = _panels_oxo(Wk.T, FT, 128)
    wo_p = _panels_oxo(Wo.T, FT, 128)
    wv_p = _panels_oxo(Wv.T, 3, 512)

    ep = np.zeros((D, ECP), np.float32)
    for c in range(NCOMP):
        ep[:, c * EP:c * EP + E] = ehs[c].T
    eT_dev = np.ascontiguousarray(
        ep.reshape(FT, 128, ECP).transpose(1, 0, 2)
        .reshape(128, FT * ECP)).astype(ml_dtypes.bfloat16)

    bo_t = np.ascontiguousarray(bo_v.reshape(FT, 128).T)

    nc = _get_nc()
    in_maps = []
    for i in range(NCORES):
        x_i = np.ascontiguousarray(
            hs[:, i * SL:(i + 1) * SL, :].reshape(NCOMP, SL, FT, 128)
            .transpose(0, 3, 2, 1).reshape(NCOMP, 128, FT * SL)
        ).astype(ml_dtypes.bfloat16)
        in_maps.append({
            "x": x_i, "eT": eT_dev, "wq": wq_p, "wk": wk_p,
            "wv": wv_p, "wo": wo_p, "bo": bo_t,
        })

    res = run_bass_kernel_spmd(nc, in_maps, core_ids=list(range(NCORES)))

    out = np.empty((NCOMP, S, D), dtype=np.float32)
    for i in range(NCORES):
        o = res.results[i]["outT"].reshape(NCOMP, 128, FT, SL)
        out[:, i * SL:(i + 1) * SL, :] = o.transpose(0, 3, 2, 1).reshape(
            NCOMP, SL, D)
    return out


# revision 10
# speedup vs baseline: 1.6730x; 1.3411x over previous
"""Trainium2 Bass kernel for DecomposingAttnProcessor (pad variant), v3.

Math (pad branch contributes exactly zero since pad tokens are zeros
projected with no bias -> k_pad = v_pad = 0):
    q = hs @ Wq.T / (temp + eps)   (scale folded into Wq on host)
    k = ehs @ Wk.T ; v = ehs @ Wv.T
    scores[c,h,s,e] = q . k        (per head, dh=64)
    w = softmax over the 4 components c (dim 0)
    o = w @ v ; out = o @ Wo.T + bo + hs

Sharding: 8 cores, split S=4096 into 512-row blocks; all 4 components of
a block stay on one core (softmax couples them). K/V computed redundantly
per core (encoder seq is only 154).

v3 design:
 - one full 512-column pass over the s-block (N=512 matmuls, Wq/Wo one load)
 - E padded to 160 per component host-side (zeros); the e-tails (e=128..154)
   of the 4 components are packed at 32-partition offsets of shared tiles
   via matmul tile_position; tail block-sum over components and the
   block-broadcast of the reciprocal run on the TENSOR engine using constant
   block-identity matrices (walrus forbids unequal base partitions on
   two-SBUF-input DVE ops, so the cross-block work must go through PE)
 - per-component score psum is one [128,1024] 2-bank tile (both heads), so
   exp is a single ACT per component
 - softmax: exp+q-copies on ScalarE, sums/recip(reciprocal_approx_fast)/
   tail-mul on DVE, main weight muls on GpSimd
 - o-matmuls of hp-1 emitted after scores of hp (SW pipeline) so the
   in-order tensor stream never stalls on the softmax chain
 - all DRAM tensors host-arranged so every DMA is dense per partition
"""

import numpy as np
import ml_dtypes

import concourse.bass as bass
import concourse.mybir as mybir
import concourse.tile as tile
from concourse import bacc
from concourse.bass_utils import run_bass_kernel_spmd

F32 = mybir.dt.float32
BF16 = mybir.dt.bfloat16
AF = mybir.ActivationFunctionType
ALU = mybir.AluOpType

NCOMP = 4
HEADS = 24
DH = 64
D = 1536
S = 4096
E = 154
EP = 160                  # padded per-component encoder length
ECP = NCOMP * EP          # 640
EPS = 1e-8
NCORES = 8
SL = S // NCORES          # 512 s-rows per core (per component)
FT = D // 128             # 12 feature tiles of 128
HP = HEADS // 2           # 12 head-pairs (2 heads = 128 feature rows)


def _emit(tc):
    nc = tc.nc

    x = nc.declare_dram_parameter("x", [NCOMP, 128, FT * SL], BF16,
                                  isOutput=False)
    eT = nc.declare_dram_parameter("eT", [128, FT * ECP], BF16, isOutput=False)
    wq = nc.declare_dram_parameter("wq", [FT, 128, FT * 128], BF16,
                                   isOutput=False)
    wk = nc.declare_dram_parameter("wk", [FT, 128, FT * 128], BF16,
                                   isOutput=False)
    wv = nc.declare_dram_parameter("wv", [3, 128, FT * 512], BF16,
                                   isOutput=False)
    wo = nc.declare_dram_parameter("wo", [FT, 128, FT * 128], BF16,
                                   isOutput=False)
    bo = nc.declare_dram_parameter("bo", [128, FT], F32, isOutput=False)
    onesA = nc.declare_dram_parameter("onesA", [128, 32], BF16, isOutput=False)
    onesB = nc.declare_dram_parameter("onesB", [32, 128], BF16, isOutput=False)
    outT = nc.declare_dram_parameter("outT", [NCOMP, 128, FT * SL], F32,
                                     isOutput=True)

    with tc.tile_pool(name="persist", bufs=1) as pp:
        # ---------------- persistent tiles ----------------
        kt_sb = [pp.tile([128, ECP], BF16, tag="kT", bufs=FT, name=f"kt{t}")
                 for t in range(FT)]
        v_sb = [pp.tile([128, D], BF16, tag="v", bufs=NCOMP, name=f"v{c}")
                for c in range(NCOMP)]
        vt_sb = pp.tile([128, D], BF16, tag="vt", bufs=1, name="vt")
        x_sb = [pp.tile([128, FT * SL], BF16, tag="x", bufs=NCOMP,
                        name=f"x{c}")
                for c in range(NCOMP)]
        bo_sb = pp.tile([128, FT], F32, tag="bo", bufs=1, name="bo_sb")
        oa_sb = pp.tile([128, 32], BF16, tag="oa", bufs=1, name="oa_sb")
        ob_sb = pp.tile([32, 128], BF16, tag="ob1", bufs=1, name="ob_sb")

        nc.sync.dma_start(out=bo_sb[:], in_=bo[:])
        nc.sync.dma_start(out=oa_sb[:], in_=onesA[:])
        nc.sync.dma_start(out=ob_sb[:], in_=onesB[:])
        for c in range(NCOMP):
            nc.sync.dma_start(out=x_sb[c][:], in_=x[c])

        # ---------------- phase A: K^T and V ----------------
        with (
            tc.tile_pool(name="pha", bufs=1) as pa,
            tc.tile_pool(name="pha_psum", bufs=1, space="PSUM") as pap,
        ):
            et_b = pa.tile([128, FT * ECP], BF16, tag="eT", bufs=1,
                           name="et_b")
            nc.sync.dma_start(out=et_b[:], in_=eT[:])

            # K^T[fo, c*EP + e] over fi; N split 320+320
            for fot in range(FT):
                wk_b = pa.tile([128, FT * 128], BF16, tag="wk", bufs=3,
                               name=f"wk{fot}")
                nc.sync.dma_start(out=wk_b[:], in_=wk[fot])
                for nch in range(2):
                    n0 = nch * 320
                    pk = pap.tile([128, 320], F32, tag="pk", bufs=2,
                                  name=f"pk{fot}_{nch}")
                    for fi in range(FT):
                        nc.tensor.matmul(
                            pk[:], wk_b[:, fi * 128:(fi + 1) * 128],
                            et_b[:, fi * ECP + n0:fi * ECP + n0 + 320],
                            start=(fi == 0), stop=(fi == FT - 1))
                    nc.vector.tensor_copy(
                        out=kt_sb[fot][:, n0:n0 + 320], in_=pk[:])

            # V[c][e, fv] (e on partitions); the four e-tails are packed at
            # partition offsets 32c of one shared tile (pad rows come out
            # zero because the eT pad columns are zero)
            for fvc in range(3):
                wv_b = pa.tile([128, FT * 512], BF16, tag="wv", bufs=2,
                               name=f"wv{fvc}")
                nc.sync.dma_start(out=wv_b[:], in_=wv[fvc])
                for c in range(NCOMP):
                    pv = pap.tile([128, 512], F32, tag="pv", bufs=2,
                                  name=f"pv{fvc}_{c}")
                    for fi in range(FT):
                        nc.tensor.matmul(
                            pv[:],
                            et_b[:, fi * ECP + c * EP:fi * ECP + c * EP + 128],
                            wv_b[:, fi * 512:(fi + 1) * 512],
                            start=(fi == 0), stop=(fi == FT - 1))
                    nc.vector.tensor_copy(
                        out=v_sb[c][:, fvc * 512:(fvc + 1) * 512], in_=pv[:])
                pvt = pap.tile([128, 512], F32, tag="pv", bufs=2,
                               name=f"pvt{fvc}")
                for c in range(NCOMP):
                    for fi in range(FT):
                        nc.tensor.matmul(
                            pvt[32 * c:32 * c + 32, :],
                            et_b[:, fi * ECP + c * EP + 128:
                                 fi * ECP + c * EP + 160],
                            wv_b[:, fi * 512:(fi + 1) * 512],
                            start=(fi == 0), stop=(fi == FT - 1),
                            skip_group_check=True, tile_position=(0, 32 * c))
                nc.vector.tensor_copy(
                    out=vt_sb[:, fvc * 512:(fvc + 1) * 512], in_=pvt[:])

        # ---------------- phases B+C ----------------
        with (
            tc.tile_pool(name="bc", bufs=1) as bc,
            tc.tile_pool(name="bcp", bufs=1, space="PSUM") as bcp,
        ):
            ot_sb = {}

            def emit_o(hp, wex, wt):
                """o-matmuls + psum evacuation for head-pair hp."""
                for c in range(NCOMP):
                    po = bcp.tile([128, 512], F32, tag="po", bufs=2,
                                  name=f"po{hp}_{c}")
                    for hh in range(2):
                        h = hp * 2 + hh
                        nc.tensor.matmul(
                            po[hh * 64:(hh + 1) * 64, :],
                            v_sb[c][:, h * 64:(h + 1) * 64],
                            wex[c][:, hh * 512:(hh + 1) * 512],
                            start=True, stop=False, skip_group_check=True)
                        nc.tensor.matmul(
                            po[hh * 64:(hh + 1) * 64, :],
                            vt_sb[32 * c:32 * c + 32, h * 64:(h + 1) * 64],
                            wt[32 * c:32 * c + 32, hh * 512:(hh + 1) * 512],
                            start=False, stop=True, skip_group_check=True,
                            tile_position=(32 * c, hh * 64))
                    ot = bc.tile([128, 512], BF16, tag="ot", bufs=48,
                                 name=f"ot{hp}_{c}")
                    nc.vector.tensor_copy(out=ot[:], in_=po[:])
                    ot_sb[(c, hp)] = ot

            prev = None
            for hp in range(HP):
                wq_b = bc.tile([128, FT * 128], BF16, tag="wq", bufs=2,
                               name=f"wq{hp}")
                nc.sync.dma_start(out=wq_b[:], in_=wq[hp])

                # Q^T for the two heads of this pair, all 4 components
                qt = []
                for c in range(NCOMP):
                    pq = bcp.tile([128, 512], F32, tag="pq", bufs=2,
                                  name=f"pq{hp}_{c}")
                    for fi in range(FT):
                        nc.tensor.matmul(
                            pq[:], wq_b[:, fi * 128:(fi + 1) * 128],
                            x_sb[c][:, fi * 512:(fi + 1) * 512],
                            start=(fi == 0), stop=(fi == FT - 1))
                    q = bc.tile([128, 512], BF16, tag="qt", bufs=4,
                                name=f"qt{hp}_{c}")
                    nc.scalar.copy(q[:], pq[:])
                    qt.append(q)

                # scores + exp; both heads share one 2-bank psum per comp
                ex = []
                for c in range(NCOMP):
                    ps = bcp.tile([128, 1024], F32, tag="ps", bufs=2,
                                  name=f"ps{hp}_{c}")
                    for hh in range(2):
                        nc.tensor.matmul(
                            ps[:, hh * 512:(hh + 1) * 512],
                            kt_sb[hp][hh * 64:(hh + 1) * 64,
                                      c * EP:c * EP + 128],
                            qt[c][hh * 64:(hh + 1) * 64, :],
                            start=True, stop=True, skip_group_check=True)
                    e = bc.tile([128, 1024], BF16, tag="ex", bufs=5,
                                name=f"ex{hp}_{c}")
                    nc.scalar.activation(e[:], ps[:], AF.Exp)
                    ex.append(e)
                # tails: 4 comps packed at partition offsets 32c
                pst = bcp.tile([128, 1024], F32, tag="ps", bufs=2,
                               name=f"pst{hp}")
                for hh in range(2):
                    for c in range(NCOMP):
                        nc.tensor.matmul(
                            pst[32 * c:32 * c + 32,
                                hh * 512:(hh + 1) * 512],
                            kt_sb[hp][hh * 64:(hh + 1) * 64,
                                      c * EP + 128:c * EP + 160],
                            qt[c][hh * 64:(hh + 1) * 64, :],
                            start=True, stop=True, skip_group_check=True,
                            tile_position=(hh * 64, 32 * c))
                ext = bc.tile([128, 1024], BF16, tag="ext", bufs=2,
                              name=f"ext{hp}")
                nc.scalar.activation(ext[:], pst[:], AF.Exp)

                # main softmax sums + reciprocal
                s01 = bc.tile([128, 1024], BF16, tag="ssum", bufs=3,
                              name=f"s01_{hp}")
                nc.vector.tensor_add(out=s01[:], in0=ex[0][:], in1=ex[1][:])
                s23 = bc.tile([128, 1024], BF16, tag="ssum", bufs=3,
                              name=f"s23_{hp}")
                nc.vector.tensor_add(out=s23[:], in0=ex[2][:], in1=ex[3][:])
                sf = bc.tile([128, 1024], F32, tag="sf", bufs=1,
                             name=f"sf{hp}")
                nc.vector.tensor_add(out=sf[:], in0=s01[:], in1=s23[:])
                rf = bc.tile([128, 1024], F32, tag="rf", bufs=2,
                             name=f"rf{hp}")
                nc.vector.reciprocal_approx_fast(out=rf[:], in_=sf[:])
                rb = bc.tile([128, 1024], BF16, tag="rb", bufs=2,
                             name=f"rb{hp}")
                nc.vector.tensor_copy(out=rb[:], in_=rf[:])

                # tail block-sum over comps / reciprocal / block-broadcast:
                # cross-partition-block work runs on PE via block-identities
                pd = bcp.tile([32, 1024], F32, tag="ps", bufs=2,
                              name=f"pd{hp}")
                for hh in range(2):
                    nc.tensor.matmul(
                        pd[:, hh * 512:(hh + 1) * 512], oa_sb[:],
                        ext[:, hh * 512:(hh + 1) * 512],
                        start=True, stop=True, skip_group_check=True)
                rtf = bc.tile([32, 1024], F32, tag="rf", bufs=2,
                              name=f"rtf{hp}")
                nc.vector.reciprocal_approx_fast(out=rtf[:], in_=pd[:])
                rtb = bc.tile([32, 1024], BF16, tag="rb", bufs=2,
                              name=f"rtb{hp}")
                nc.vector.tensor_copy(out=rtb[:], in_=rtf[:])
                pr = bcp.tile([128, 1024], F32, tag="ps", bufs=2,
                              name=f"pr{hp}")
                for hh in range(2):
                    nc.tensor.matmul(
                        pr[:, hh * 512:(hh + 1) * 512], ob_sb[:],
                        rtb[:, hh * 512:(hh + 1) * 512],
                        start=True, stop=True, skip_group_check=True)

                # normalized weights: main on GpSimd, packed tail on DVE
                wex = []
                for c in range(NCOMP):
                    w = bc.tile([128, 1024], BF16, tag="wex", bufs=5,
                                name=f"wex{hp}_{c}")
                    nc.gpsimd.tensor_mul(out=w[:], in0=ex[c][:], in1=rb[:])
                    wex.append(w)
                wt = bc.tile([128, 1024], BF16, tag="wt", bufs=2,
                             name=f"wt{hp}")
                nc.vector.tensor_mul(out=wt[:], in0=ext[:], in1=pr[:])

                # SW pipeline: previous hp's o-matmuls issue after this hp's
                # Q/scores so the tensor stream doesn't stall on softmax
                if prev is not None:
                    emit_o(*prev)
                prev = (hp, wex, wt)
            emit_o(*prev)

            # -------- phase C: out-proj + bias + residual --------
            for fot in range(FT):
                wo_b = bc.tile([128, FT * 128], BF16, tag="wq", bufs=2,
                               name=f"wo{fot}")
                nc.sync.dma_start(out=wo_b[:], in_=wo[fot])
                for c in range(NCOMP):
                    pco = bcp.tile([128, 512], F32, tag="po", bufs=2,
                                   name=f"pc{fot}_{c}")
                    for fi in range(FT):
                        nc.tensor.matmul(
                            pco[:], wo_b[:, fi * 128:(fi + 1) * 128],
                            ot_sb[(c, fi)][:],
                            start=(fi == 0), stop=(fi == FT - 1))
                    ob = bc.tile([128, 512], F32, tag="ob", bufs=4,
                                 name=f"ob{fot}_{c}")
                    nc.vector.scalar_tensor_tensor(
                        out=ob[:], in0=pco[:],
                        scalar=bo_sb[:, fot:fot + 1],
                        in1=x_sb[c][:, fot * 512:(fot + 1) * 512],
                        op0=ALU.add, op1=ALU.add)
                    nc.sync.dma_start(
                        out=outT[c][:, fot * 512:(fot + 1) * 512], in_=ob[:])


_NC_CACHE = {}


def _get_nc():
    if "nc" not in _NC_CACHE:
        nc = bacc.Bacc("TRN2", target_bir_lowering=False)
        with tile.TileContext(nc) as tc:
            _emit(tc)
        nc.compile()
        _NC_CACHE["nc"] = nc
    return _NC_CACHE["nc"]


def _panels_oxo(wT, nblk, blk):
    """[D, D] -> [nblk, 128, FT*blk]: panel[b][p][fi*blk+o] = wT[fi*128+p, b*blk+o]."""
    return np.ascontiguousarray(
        wT.reshape(FT, 128, nblk, blk).transpose(2, 1, 0, 3)
        .reshape(nblk, 128, FT * blk)).astype(ml_dtypes.bfloat16)


def kernel(hidden_states, encoder_hidden_states, temperature, Wq, Wk, Wv, Wo,
           bo, pad_length):
    # pad branch contributes zero to the output (zeros projected with no
    # bias give k_pad = v_pad = 0), so pad_length is irrelevant.
    hs = np.asarray(hidden_states, dtype=np.float32)
    ehs = np.asarray(encoder_hidden_states, dtype=np.float32)
    temp = float(np.asarray(temperature).reshape(-1)[0])
    Wq = np.asarray(Wq, dtype=np.float32)
    Wk = np.asarray(Wk, dtype=np.float32)
    Wv = np.asarray(Wv, dtype=np.float32)
    Wo = np.asarray(Wo, dtype=np.float32)
    bo_v = np.asarray(bo, dtype=np.float32).reshape(-1)

    wq_p = _panels_oxo((Wq / (temp + EPS)).T, FT, 128)
    wk_p = _panels_oxo(Wk.T, FT, 128)
    wo_p = _panels_oxo(Wo.T, FT, 128)
    wv_p = _panels_oxo(Wv.T, 3, 512)

    ep = np.zeros((D, ECP), np.float32)
    for c in range(NCOMP):
        ep[:, c * EP:c * EP + E] = ehs[c].T
    eT_dev = np.ascontiguousarray(
        ep.reshape(FT, 128, ECP).transpose(1, 0, 2)
        .reshape(128, FT * ECP)).astype(ml_dtypes.bfloat16)

    bo_t = np.ascontiguousarray(bo_v.reshape(FT, 128).T)

    onesA = np.zeros((128, 32), np.float32)
    for k in range(128):
        onesA[k, k % 32] = 1.0
    onesB = np.zeros((32, 128), np.float32)
    for m in range(128):
        onesB[m % 32, m] = 1.0
    onesA = onesA.astype(ml_dtypes.bfloat16)
    onesB = onesB.astype(ml_dtypes.bfloat16)

    nc = _get_nc()
    in_maps = []
    for i in range(NCORES):
        x_i = np.ascontiguousarray(
            hs[:, i * SL:(i + 1) * SL, :].reshape(NCOMP, SL, FT, 128)
            .transpose(0, 3, 2, 1).reshape(NCOMP, 128, FT * SL)
        ).astype(ml_dtypes.bfloat16)
        in_maps.append({
            "x": x_i, "eT": eT_dev, "wq": wq_p, "wk": wk_p,
            "wv": wv_p, "wo": wo_p, "bo": bo_t,
            "onesA": onesA, "onesB": onesB,
        })

    res = run_bass_kernel_spmd(nc, in_maps, core_ids=list(range(NCORES)))

    out = np.empty((NCOMP, S, D), dtype=np.float32)
    for i in range(NCORES):
        o = res.results[i]["outT"].reshape(NCOMP, 128, FT, SL)
        out[:, i * SL:(i + 1) * SL, :] = o.transpose(0, 3, 2, 1).reshape(
            NCOMP, SL, D)
    return out


# revision 12
# speedup vs baseline: 1.7669x; 1.0561x over previous
"""Trainium2 Bass kernel for DecomposingAttnProcessor (pad variant), v3.

Math (pad branch contributes exactly zero since pad tokens are zeros
projected with no bias -> k_pad = v_pad = 0):
    q = hs @ Wq.T / (temp + eps)   (scale folded into Wq on host)
    k = ehs @ Wk.T ; v = ehs @ Wv.T
    scores[c,h,s,e] = q . k        (per head, dh=64)
    w = softmax over the 4 components c (dim 0)
    o = w @ v ; out = o @ Wo.T + bo + hs

Sharding: 8 cores, split S=4096 into 512-row blocks; all 4 components of
a block stay on one core (softmax couples them). K/V computed redundantly
per core (encoder seq is only 154).

v3 design:
 - one full 512-column pass over the s-block (N=512 matmuls, Wq/Wo one load)
 - E padded to 160 per component host-side (zeros); the e-tails (e=128..154)
   of the 4 components are packed at 32-partition offsets of shared tiles
   via matmul tile_position; tail block-sum over components and the
   block-broadcast of the reciprocal run on the TENSOR engine using constant
   block-identity matrices (walrus forbids unequal base partitions on
   two-SBUF-input DVE ops, so the cross-block work must go through PE)
 - per-component score psum is one [128,1024] 2-bank tile (both heads), so
   exp is a single ACT per component
 - softmax: exp+q-copies on ScalarE, sums/recip(reciprocal_approx_fast)/
   tail-mul on DVE, main weight muls on GpSimd
 - o-matmuls of hp-1 emitted after scores of hp (SW pipeline) so the
   in-order tensor stream never stalls on the softmax chain
 - all DRAM tensors host-arranged so every DMA is dense per partition
"""

import numpy as np
import ml_dtypes

import concourse.bass as bass
import concourse.mybir as mybir
import concourse.tile as tile
from concourse import bacc
from concourse.bass_utils import run_bass_kernel_spmd

F32 = mybir.dt.float32
BF16 = mybir.dt.bfloat16
AF = mybir.ActivationFunctionType
ALU = mybir.AluOpType

NCOMP = 4
HEADS = 24
DH = 64
D = 1536
S = 4096
E = 154
EP = 160                  # padded per-component encoder length
ECP = NCOMP * EP          # 640
EPS = 1e-8
NCORES = 8
SL = S // NCORES          # 512 s-rows per core (per component)
FT = D // 128             # 12 feature tiles of 128
HP = HEADS // 2           # 12 head-pairs (2 heads = 128 feature rows)


def _emit(tc):
    nc = tc.nc

    x = nc.declare_dram_parameter("x", [NCOMP, 128, FT * SL], BF16,
                                  isOutput=False)
    eT = nc.declare_dram_parameter("eT", [128, FT * ECP], BF16, isOutput=False)
    wq = nc.declare_dram_parameter("wq", [FT, 128, FT * 128], BF16,
                                   isOutput=False)
    wk = nc.declare_dram_parameter("wk", [FT, 128, FT * 128], BF16,
                                   isOutput=False)
    wv = nc.declare_dram_parameter("wv", [3, 128, FT * 512], BF16,
                                   isOutput=False)
    wo = nc.declare_dram_parameter("wo", [FT, 128, FT * 128], BF16,
                                   isOutput=False)
    bo = nc.declare_dram_parameter("bo", [128, FT], F32, isOutput=False)
    onesA = nc.declare_dram_parameter("onesA", [128, 32], BF16, isOutput=False)
    onesB = nc.declare_dram_parameter("onesB", [32, 128], BF16, isOutput=False)
    outT = nc.declare_dram_parameter("outT", [NCOMP, 128, FT * SL], F32,
                                     isOutput=True)

    with (
        tc.tile_pool(name="persist", bufs=1) as pp,
        tc.tile_pool(name="psum", bufs=1, space="PSUM") as px,
    ):
        # ---------------- persistent tiles ----------------
        kt_sb = [pp.tile([128, ECP], BF16, tag="kT", bufs=FT, name=f"kt{t}")
                 for t in range(FT)]
        v_sb = [pp.tile([128, D], BF16, tag="v", bufs=NCOMP, name=f"v{c}")
                for c in range(NCOMP)]
        vt_sb = pp.tile([128, D], BF16, tag="vt", bufs=1, name="vt")
        x_sb = [pp.tile([128, FT * SL], BF16, tag="x", bufs=NCOMP,
                        name=f"x{c}")
                for c in range(NCOMP)]
        bo_sb = pp.tile([128, FT], F32, tag="bo", bufs=1, name="bo_sb")
        oa_sb = pp.tile([128, 32], BF16, tag="oa", bufs=1, name="oa_sb")
        ob_sb = pp.tile([32, 128], BF16, tag="ob1", bufs=1, name="ob_sb")

        nc.sync.dma_start(out=bo_sb[:], in_=bo[:])
        nc.sync.dma_start(out=oa_sb[:], in_=onesA[:])
        nc.sync.dma_start(out=ob_sb[:], in_=onesB[:])
        for c in range(NCOMP):
            nc.sync.dma_start(out=x_sb[c][:], in_=x[c])

        # ---------------- phase A: K^T and V ----------------
        with tc.tile_pool(name="pha", bufs=1) as pa:
            et_b = pa.tile([128, FT * ECP], BF16, tag="eT", bufs=1,
                           name="et_b")
            nc.sync.dma_start(out=et_b[:], in_=eT[:])

            # K^T[fo, c*EP + e] over fi; N split 320+320
            for fot in range(FT):
                wk_b = pa.tile([128, FT * 128], BF16, tag="wk", bufs=3,
                               name=f"wk{fot}")
                nc.sync.dma_start(out=wk_b[:], in_=wk[fot])
                for nch in range(2):
                    n0 = nch * 320
                    pk = px.tile([128, 320], F32, tag="po", bufs=2,
                                  name=f"pk{fot}_{nch}")
                    for fi in range(FT):
                        nc.tensor.matmul(
                            pk[:], wk_b[:, fi * 128:(fi + 1) * 128],
                            et_b[:, fi * ECP + n0:fi * ECP + n0 + 320],
                            start=(fi == 0), stop=(fi == FT - 1))
                    nc.vector.tensor_copy(
                        out=kt_sb[fot][:, n0:n0 + 320], in_=pk[:])

            # V[c][e, fv] (e on partitions); the four e-tails are packed at
            # partition offsets 32c of one shared tile (pad rows come out
            # zero because the eT pad columns are zero)
            for fvc in range(3):
                wv_b = pa.tile([128, FT * 512], BF16, tag="wv", bufs=2,
                               name=f"wv{fvc}")
                nc.sync.dma_start(out=wv_b[:], in_=wv[fvc])
                for c in range(NCOMP):
                    pv = px.tile([128, 512], F32, tag="po", bufs=2,
                                  name=f"pv{fvc}_{c}")
                    for fi in range(FT):
                        nc.tensor.matmul(
                            pv[:],
                            et_b[:, fi * ECP + c * 128:fi * ECP + c * 128 + 128],
                            wv_b[:, fi * 512:(fi + 1) * 512],
                            start=(fi == 0), stop=(fi == FT - 1))
                    nc.vector.tensor_copy(
                        out=v_sb[c][:, fvc * 512:(fvc + 1) * 512], in_=pv[:])
                pvt = px.tile([128, 512], F32, tag="po", bufs=2,
                               name=f"pvt{fvc}")
                for fi in range(FT):
                    nc.tensor.matmul(
                        pvt[:],
                        et_b[:, fi * ECP + 512:fi * ECP + 640],
                        wv_b[:, fi * 512:(fi + 1) * 512],
                        start=(fi == 0), stop=(fi == FT - 1))
                nc.vector.tensor_copy(
                    out=vt_sb[:, fvc * 512:(fvc + 1) * 512], in_=pvt[:])

        # ---------------- phases B+C ----------------
        with tc.tile_pool(name="bc", bufs=1) as bc:
            ot_sb = {}

            def emit_o(hp, wex, wt):
                """o-matmuls + psum evacuation for head-pair hp."""
                for c in range(NCOMP):
                    po = px.tile([128, 512], F32, tag="po", bufs=2,
                                  name=f"po{hp}_{c}")
                    for hh in range(2):
                        h = hp * 2 + hh
                        nc.tensor.matmul(
                            po[hh * 64:(hh + 1) * 64, :],
                            v_sb[c][:, h * 64:(h + 1) * 64],
                            wex[c][:, hh * 512:(hh + 1) * 512],
                            start=True, stop=False, skip_group_check=True)
                        nc.tensor.matmul(
                            po[hh * 64:(hh + 1) * 64, :],
                            vt_sb[32 * c:32 * c + 32, h * 64:(h + 1) * 64],
                            wt[32 * c:32 * c + 32, hh * 512:(hh + 1) * 512],
                            start=False, stop=True, skip_group_check=True,
                            tile_position=(32 * c, hh * 64))
                    ot = bc.tile([128, 512], BF16, tag="ot", bufs=48,
                                 name=f"ot{hp}_{c}")
                    nc.vector.tensor_copy(out=ot[:], in_=po[:])
                    ot_sb[(c, hp)] = ot

            prev = None
            for hp in range(HP):
                wq_b = pp.tile([128, FT * 128], BF16, tag="wq", bufs=2,
                               name=f"wq{hp}")
                nc.sync.dma_start(out=wq_b[:], in_=wq[hp])

                # Q^T for the two heads of this pair, all 4 components
                qt = []
                for c in range(NCOMP):
                    pq = px.tile([128, 512], F32, tag="pq", bufs=2,
                                  name=f"pq{hp}_{c}")
                    for fi in range(FT):
                        nc.tensor.matmul(
                            pq[:], wq_b[:, fi * 128:(fi + 1) * 128],
                            x_sb[c][:, fi * 512:(fi + 1) * 512],
                            start=(fi == 0), stop=(fi == FT - 1))
                    q = pp.tile([128, 512], BF16, tag="qt", bufs=4,
                                name=f"qt{hp}_{c}")
                    nc.scalar.copy(q[:], pq[:])
                    qt.append(q)

                # scores + exp; both heads share one 2-bank psum per comp
                ex = []
                for c in range(NCOMP):
                    ps = px.tile([128, 1024], F32, tag="ps", bufs=2,
                                  name=f"ps{hp}_{c}")
                    for hh in range(2):
                        nc.tensor.matmul(
                            ps[:, hh * 512:(hh + 1) * 512],
                            kt_sb[hp][hh * 64:(hh + 1) * 64,
                                      c * 128:c * 128 + 128],
                            qt[c][hh * 64:(hh + 1) * 64, :],
                            start=True, stop=True, skip_group_check=True)
                    e = bc.tile([128, 1024], BF16, tag="ex", bufs=5,
                                name=f"ex{hp}_{c}")
                    nc.scalar.activation(e[:], ps[:], AF.Exp)
                    ex.append(e)
                # tails: 4 comps packed at partition offsets 32c
                pst = px.tile([128, 1024], F32, tag="ps", bufs=2,
                               name=f"pst{hp}")
                for hh in range(2):
                    for c in range(NCOMP):
                        nc.tensor.matmul(
                            pst[32 * c:32 * c + 32,
                                hh * 512:(hh + 1) * 512],
                            kt_sb[hp][hh * 64:(hh + 1) * 64,
                                      512 + 32 * c:512 + 32 * c + 32],
                            qt[c][hh * 64:(hh + 1) * 64, :],
                            start=True, stop=True, skip_group_check=True,
                            tile_position=(hh * 64, 32 * c))
                ext = bc.tile([128, 1024], BF16, tag="ext", bufs=2,
                              name=f"ext{hp}")
                nc.scalar.activation(ext[:], pst[:], AF.Exp)

                # main softmax sums + reciprocal
                s01 = bc.tile([128, 1024], BF16, tag="ssum", bufs=3,
                              name=f"s01_{hp}")
                nc.vector.tensor_add(out=s01[:], in0=ex[0][:], in1=ex[1][:])
                s23 = bc.tile([128, 1024], BF16, tag="ssum", bufs=3,
                              name=f"s23_{hp}")
                nc.vector.tensor_add(out=s23[:], in0=ex[2][:], in1=ex[3][:])
                sf = bc.tile([128, 1024], F32, tag="sf", bufs=1,
                             name=f"sf{hp}")
                nc.vector.tensor_add(out=sf[:], in0=s01[:], in1=s23[:])
                rf = bc.tile([128, 1024], F32, tag="rf", bufs=2,
                             name=f"rf{hp}")
                nc.vector.reciprocal_approx_fast(out=rf[:], in_=sf[:])
                rb = bc.tile([128, 1024], BF16, tag="rb", bufs=2,
                             name=f"rb{hp}")
                nc.vector.tensor_copy(out=rb[:], in_=rf[:])

                # tail block-sum over comps / reciprocal / block-broadcast:
                # cross-partition-block work runs on PE via block-identities
                pd = px.tile([32, 1024], F32, tag="ps", bufs=2,
                              name=f"pd{hp}")
                for hh in range(2):
                    nc.tensor.matmul(
                        pd[:, hh * 512:(hh + 1) * 512], oa_sb[:],
                        ext[:, hh * 512:(hh + 1) * 512],
                        start=True, stop=True, skip_group_check=True)
                rtf = bc.tile([32, 1024], F32, tag="rf", bufs=2,
                              name=f"rtf{hp}")
                nc.vector.reciprocal_approx_fast(out=rtf[:], in_=pd[:])
                rtb = bc.tile([32, 1024], BF16, tag="rb", bufs=2,
                              name=f"rtb{hp}")
                nc.vector.tensor_copy(out=rtb[:], in_=rtf[:])
                pr = px.tile([128, 1024], F32, tag="ps", bufs=2,
                              name=f"pr{hp}")
                for hh in range(2):
                    nc.tensor.matmul(
                        pr[:, hh * 512:(hh + 1) * 512], ob_sb[:],
                        rtb[:, hh * 512:(hh + 1) * 512],
                        start=True, stop=True, skip_group_check=True)

                # normalized weights: main on GpSimd, packed tail on DVE
                wex = []
                for c in range(NCOMP):
                    w = bc.tile([128, 1024], BF16, tag="wex", bufs=5,
                                name=f"wex{hp}_{c}")
                    nc.gpsimd.tensor_mul(out=w[:], in0=ex[c][:], in1=rb[:])
                    wex.append(w)
                wt = bc.tile([128, 1024], BF16, tag="wt", bufs=2,
                             name=f"wt{hp}")
                nc.vector.tensor_mul(out=wt[:], in0=ext[:], in1=pr[:])

                # SW pipeline: previous hp's o-matmuls issue after this hp's
                # Q/scores so the tensor stream doesn't stall on softmax
                if prev is not None:
                    emit_o(*prev)
                prev = (hp, wex, wt)
            emit_o(*prev)

            # -------- phase C: out-proj + bias + residual --------
            for fot in range(FT):
                wo_b = pp.tile([128, FT * 128], BF16, tag="wq", bufs=2,
                               name=f"wo{fot}")
                nc.sync.dma_start(out=wo_b[:], in_=wo[fot])
                for c in range(NCOMP):
                    pco = px.tile([128, 512], F32, tag="pq", bufs=2,
                                   name=f"pc{fot}_{c}")
                    for fi in range(FT):
                        nc.tensor.matmul(
                            pco[:], wo_b[:, fi * 128:(fi + 1) * 128],
                            ot_sb[(c, fi)][:],
                            start=(fi == 0), stop=(fi == FT - 1))
                    ob = bc.tile([128, 512], F32, tag="ob", bufs=4,
                                 name=f"ob{fot}_{c}")
                    nc.vector.scalar_tensor_tensor(
                        out=ob[:], in0=pco[:],
                        scalar=bo_sb[:, fot:fot + 1],
                        in1=x_sb[c][:, fot * 512:(fot + 1) * 512],
                        op0=ALU.add, op1=ALU.add)
                    nc.sync.dma_start(
                        out=outT[c][:, fot * 512:(fot + 1) * 512], in_=ob[:])


_NC_CACHE = {}


def _get_nc():
    if "nc" not in _NC_CACHE:
        nc = bacc.Bacc("TRN2", target_bir_lowering=False)
        with tile.TileContext(nc) as tc:
            _emit(tc)
        nc.compile()
        _NC_CACHE["nc"] = nc
    return _NC_CACHE["nc"]


def _panels_oxo(wT, nblk, blk):
    """[D, D] -> [nblk, 128, FT*blk]: panel[b][p][fi*blk+o] = wT[fi*128+p, b*blk+o]."""
    return np.ascontiguousarray(
        wT.reshape(FT, 128, nblk, blk).transpose(2, 1, 0, 3)
        .reshape(nblk, 128, FT * blk)).astype(ml_dtypes.bfloat16)


def kernel(hidden_states, encoder_hidden_states, temperature, Wq, Wk, Wv, Wo,
           bo, pad_length):
    # pad branch contributes zero to the output (zeros projected with no
    # bias give k_pad = v_pad = 0), so pad_length is irrelevant.
    hs = np.asarray(hidden_states, dtype=np.float32)
    ehs = np.asarray(encoder_hidden_states, dtype=np.float32)
    temp = float(np.asarray(temperature).reshape(-1)[0])
    Wq = np.asarray(Wq, dtype=np.float32)
    Wk = np.asarray(Wk, dtype=np.float32)
    Wv = np.asarray(Wv, dtype=np.float32)
    Wo = np.asarray(Wo, dtype=np.float32)
    bo_v = np.asarray(bo, dtype=np.float32).reshape(-1)

    wq_p = _panels_oxo((Wq / (temp + EPS)).T, FT, 128)
    wk_p = _panels_oxo(Wk.T, FT, 128)
    wo_p = _panels_oxo(Wo.T, FT, 128)
    wv_p = _panels_oxo(Wv.T, 3, 512)

    # column layout per fi-block: [c*128+e (e<128) | 512 + c*32+j (j<26)]
    ep = np.zeros((D, ECP), np.float32)
    for c in range(NCOMP):
        ep[:, c * 128:c * 128 + 128] = ehs[c][:128].T
        ep[:, 512 + c * 32:512 + c * 32 + (E - 128)] = ehs[c][128:].T
    eT_dev = np.ascontiguousarray(
        ep.reshape(FT, 128, ECP).transpose(1, 0, 2)
        .reshape(128, FT * ECP)).astype(ml_dtypes.bfloat16)

    bo_t = np.ascontiguousarray(bo_v.reshape(FT, 128).T)

    onesA = np.zeros((128, 32), np.float32)
    for k in range(128):
        onesA[k, k % 32] = 1.0
    onesB = np.zeros((32, 128), np.float32)
    for m in range(128):
        onesB[m % 32, m] = 1.0
    onesA = onesA.astype(ml_dtypes.bfloat16)
    onesB = onesB.astype(ml_dtypes.bfloat16)

    nc = _get_nc()
    in_maps = []
    for i in range(NCORES):
        x_i = np.ascontiguousarray(
            hs[:, i * SL:(i + 1) * SL, :].reshape(NCOMP, SL, FT, 128)
            .transpose(0, 3, 2, 1).reshape(NCOMP, 128, FT * SL)
        ).astype(ml_dtypes.bfloat16)
        in_maps.append({
            "x": x_i, "eT": eT_dev, "wq": wq_p, "wk": wk_p,
            "wv": wv_p, "wo": wo_p, "bo": bo_t,
            "onesA": onesA, "onesB": onesB,
        })

    res = run_bass_kernel_spmd(nc, in_maps, core_ids=list(range(NCORES)))

    out = np.empty((NCOMP, S, D), dtype=np.float32)
    for i in range(NCORES):
        o = res.results[i]["outT"].reshape(NCOMP, 128, FT, SL)
        out[:, i * SL:(i + 1) * SL, :] = o.transpose(0, 3, 2, 1).reshape(
            NCOMP, SL, D)
    return out
